# revision 59
# baseline (speedup 1.0000x reference)
"""Trainium2 Bass kernel for the blockwise spiking network (nn_Blocks_86096914416140).

Sharding: data-parallel over batch B=32 across 8 NeuronCores (4 batches/core),
all parameters replicated, zero collectives.

Per-core structure: TWO independent batch-pipelines (2 batches each), phase-
offset so one pipeline's elementwise chain fills the other's matmul wait.
Channel-on-partition layout: partition = c % 128, free = (c_hi=4, b=2, t=32)
= 256 per pipeline. Per block n, per pipeline:

  xp   = x_blk (Act copy into PSUM) + W @ spikes_prev + vb@t0   [PE accum]
  cur  = min(q_prev,1) * xp       (refractory gate)              [VectorE]
  mem  = scan: s = beta*s + cur   (seg-reset via table)          [VectorE]
  fs   = (mem - 1) > thr_prev                                    [VectorE]
  g    = scan: s = max(invp*s, fs)  (first-spike marker scan)    [VectorE]
  spk  = (g == 1)                 (exact one-hot first spike)    [GpSimd]
  q    = g + ns                   (next gate)                    [VectorE/GpSimd]
  thr' = a'*(b*p^{t+1})           (8 per-(c_hi,b) scale-copies)  [ScalarE]
  a'   = p^32*a + invp*spiked/g_last                             [tiny mix]

The g max-scan makes (g==1) the one-hot first spike, (g>=1)+ns the
refractory gate, and 1/g_last the p-decay factor for the adaptation update.
Spikes are written as bf16 directly into the grouped output buffer (DMA'd
out as bf16, upcast on host). x is pretransposed on the host so each block
is one contiguous [128 x 2KB] DMA.

The recurrent weight streams through the PE as an exact 3-way bf16 split
(w1+w2+w3 == W to ~2^-27; spike operand is exact in bf16, so PSUM accumulates
the fp32-equivalent result). A dummy-matmul warmup ramps the PE to its full
clock; tiny chain-gated pulse matmuls hold the p-state across gaps.
"""

import numpy as np

B, C, T_LEN, T = 32, 512, 1024, 32
NB = T_LEN // T          # 32 blocks
NCORES = 8
BPC = B // NCORES        # 4 batches per core
CH = C // 128            # 4 channel tiles
FREE = CH * BPC * T      # 512 free elements per block (both pipelines)
GRP = 8                  # blocks per output DMA group
NG = NB // GRP
NP = 2                   # pipelines per core
BP = BPC // NP           # batches per pipeline
PF = CH * BP * T         # 256 free elements per pipeline

_compiled = None


def _build_program():
    import concourse.bass as bass
    import concourse.bacc as bacc
    import concourse.tile as tile
    from concourse import mybir
    from concourse._compat import with_exitstack
    from contextlib import ExitStack

    f32 = mybir.dt.float32
    bf16 = mybir.dt.bfloat16
    Alu = mybir.AluOpType

    nc = bacc.Bacc()
    xt_d = nc.declare_dram_parameter("xt", [NB, 128, FREE], f32, isOutput=False)
    NV = 3                   # weight-split terms streamed through the PE
    wt_d = nc.declare_dram_parameter("wt", [NV * 16, 128, 128], bf16,
                                     isOutput=False)
    id_d = nc.declare_dram_parameter("ident", [128, 128], f32, isOutput=False)
    betaseg_d = nc.declare_dram_parameter("betaseg", [128, PF], f32, isOutput=False)
    invpseg_d = nc.declare_dram_parameter("invpseg", [128, PF], f32, isOutput=False)
    bp1_d = nc.declare_dram_parameter("bp1", [128, PF], f32, isOutput=False)
    betat_d = nc.declare_dram_parameter("betat", [128, CH], f32, isOutput=False)
    p32_d = nc.declare_dram_parameter("p32t", [128, CH], f32, isOutput=False)
    invp_d = nc.declare_dram_parameter("invpt", [128, CH], f32, isOutput=False)
    out_d = nc.declare_dram_parameter("out", [NG, NP, 128, CH * BP * GRP * T],
                                      bf16, isOutput=True)

    @with_exitstack
    def kern(ctx: ExitStack, tc: tile.TileContext):
        consts = ctx.enter_context(tc.tile_pool(name="consts", bufs=1))
        xsp = ctx.enter_context(tc.tile_pool(name="xsp", bufs=3))
        work = ctx.enter_context(tc.tile_pool(name="work", bufs=2))
        sgp = ctx.enter_context(tc.tile_pool(name="sgp", bufs=2))
        small = ctx.enter_context(tc.tile_pool(name="small", bufs=3))
        psum = ctx.enter_context(tc.tile_pool(name="psum", bufs=2, space="PSUM"))
        psc = ctx.enter_context(tc.tile_pool(name="psc", bufs=1, space="PSUM"))

        dma = nc.sync

        wt_t = consts.tile([128, NV * 16, 128], bf16, tag="wt")
        dma.dma_start(out=wt_t[:], in_=wt_d[:].rearrange("k p m -> p k m"))
        id_t = consts.tile([128, 128], f32, tag="ident")
        dma.dma_start(out=id_t[:], in_=id_d[:])
        betaseg_t = consts.tile([128, PF], f32, tag="betaseg")
        dma.dma_start(out=betaseg_t[:], in_=betaseg_d[:])
        invpseg_t = consts.tile([128, PF], f32, tag="invpseg")
        dma.dma_start(out=invpseg_t[:], in_=invpseg_d[:])
        bp1_t = consts.tile([128, CH, BP, T], f32, tag="bp1")
        dma.dma_start(out=bp1_t[:],
                      in_=bp1_d[:].rearrange("p (c b t) -> p c b t", c=CH, b=BP))
        betat_t = consts.tile([128, CH, 1, 1], f32, tag="betat")
        dma.dma_start(out=betat_t[:],
                      in_=betat_d[:].rearrange("p (c u v) -> p c u v", u=1, v=1))
        p32_t = consts.tile([128, CH, 1, 1], f32, tag="p32t")
        dma.dma_start(out=p32_t[:],
                      in_=p32_d[:].rearrange("p (c u v) -> p c u v", u=1, v=1))
        invp_t = consts.tile([128, CH, 1, 1], f32, tag="invpt")
        dma.dma_start(out=invp_t[:],
                      in_=invp_d[:].rearrange("p (c u v) -> p c u v", u=1, v=1))

        pfull = [128, CH, BP, T]
        pcast = [128, CH, BP, 1]

        thr0_t = consts.tile(pfull, f32, tag="thr0")
        nc.vector.memset(thr0_t[:], 0.0)
        a0_t = consts.tile(pcast, f32, tag="a0")
        nc.vector.memset(a0_t[:], 0.0)

        # PE p-state warmup while const/x DMAs land
        dum_t = consts.tile([128, 128], bf16, tag="dum")
        nc.gpsimd.memset(dum_t[:], 0.0)
        dacc = psc.tile([128, 128], f32, tag="dacc")

        def dummies(k):
            for _ in range(k):
                nc.tensor.matmul(out=dacc[:], lhsT=dum_t[:], rhs=dum_t[:],
                                 start=True, stop=True)

        def pulse(ap):
            # near-free PE keep-alive: 1-column matmul gated on a chain tile,
            # so it fires mid-gap and holds the PE p-state
            nc.tensor.matmul(out=dacc[:, 0:1], lhsT=id_t[:], rhs=ap,
                             start=True, stop=True)

        dummies(45)

        def pflat(t4):
            return t4[:].rearrange("p c b t -> p (c b t)")

        # per-pipeline persistent state
        q_prev = [None, None]
        memp = [None, None]
        gp = [None, None]
        curp = [None, None]
        thr_prev = [None, None]
        a_prev = [None, None]
        vb_prev = [None, None]
        sgrp = [None, None]
        sg_old = [None, None]
        go_prev = 0

        for n in range(NB):
            gi, go = divmod(n, GRP)
            if go == 0:
                sg_old = sgrp
                sgrp = [sgp.tile([128, CH, BP, GRP, T], bf16, tag=f"sg{pp}",
                                 name=f"sg{pp}")
                        for pp in range(NP)]

            xs_t = xsp.tile([128, CH, BPC, T], f32, tag="xs")
            dma.dma_start(
                out=xs_t[:],
                in_=bass.AP(tensor=xt_d, offset=n * 128 * FREE,
                            ap=[[FREE, 128], [BPC * T, CH], [T, BPC], [1, T]]))

            last = n == NB - 1
            for pp in range(NP):
                xp = psum.tile(pfull, f32, tag=f"xp{pp}", name=f"xp{pp}")
                nc.scalar.copy(out=xp[:],
                               in_=xs_t[:, :, pp * BP:(pp + 1) * BP])

                if n > 0:
                    sg_rd = sgrp[pp] if go > 0 else sg_old[pp]
                    # First pulse: phase lock. Gating this stream on the OTHER
                    # pipeline's latest cur forces the scheduler (and the PE
                    # counting semaphore) to order that cur before this
                    # stream, so it never waits on our matmuls. Second pulse:
                    # p-state keep-alive that fires mid-gap.
                    pulse(memp[pp][:, 0, 0, 0:1])
                    pulse(gp[pp][:, 0, 0, 0:1])

                    def mms(v):
                        for ci in range(CH):
                            for cj in range(CH):
                                nc.tensor.matmul(
                                    out=xp[:, ci],
                                    lhsT=wt_t[:, v * 16 + cj * CH + ci],
                                    rhs=sg_rd[:, cj, :, go_prev],
                                    start=False,
                                    stop=(v == NV - 1 and ci == CH - 1
                                          and cj == CH - 1))

                    mms(0)
                    # vb (beta-scaled carry potential) into the t=0 columns;
                    # mid-stream so it never gates the PSUM stop matmul
                    nc.tensor.matmul(
                        out=xp[:, :, :, 0:1], lhsT=id_t[:],
                        rhs=vb_prev[pp].rearrange("p c b u -> p (c b u)"),
                        start=False, stop=False)
                    for v in range(1, NV):
                        mms(v)

                    cur_t = work.tile(pfull, f32, tag=f"cur{pp}",
                                      name=f"cur{pp}")
                    nc.vector.scalar_tensor_tensor(
                        out=pflat(cur_t), in0=pflat(q_prev[pp]), scalar=1.0,
                        in1=pflat(xp), op0=Alu.min, op1=Alu.mult)
                    curp[pp] = cur_t
                    sc_in = pflat(cur_t)

                    # thr for THIS block (from block n-1's adaptation state),
                    # deferred here so the engine queues see ops in execution
                    # order (its inputs finished a block ago)
                    thr_new = work.tile(pfull, f32, tag=f"thr{pp}",
                                        name=f"thr{pp}")
                    if pp == 0:
                        nc.vector.tensor_tensor(
                            out=thr_new[:], in0=a_prev[pp].broadcast_to(pfull),
                            in1=bp1_t[:], op=Alu.mult)
                    else:
                        for chi in range(CH):
                            for b in range(BP):
                                nc.scalar.mul(thr_new[:, chi, b],
                                              bp1_t[:, chi, b],
                                              a_prev[pp][:, chi, b])
                    thr_prev[pp] = thr_new
                else:
                    sc_in = pflat(xp)

                mem_t = work.tile(pfull, f32, tag=f"mem{pp}", name=f"mem{pp}")
                nc.vector.tensor_tensor_scan(
                    out=pflat(mem_t), data0=betaseg_t[:], data1=sc_in,
                    initial=0.0, op0=Alu.mult, op1=Alu.add)

                thr_in = thr_prev[pp] if n > 0 else thr0_t
                fs_t = work.tile(pfull, bf16, tag=f"fs{pp}", name=f"fs{pp}")
                nc.vector.scalar_tensor_tensor(
                    out=pflat(fs_t), in0=pflat(mem_t), scalar=1.0,
                    in1=thr_in[:].rearrange("p c b t -> p (c b t)"),
                    op0=Alu.subtract, op1=Alu.is_gt)

                g_t = work.tile(pfull, f32, tag=f"g{pp}", name=f"g{pp}")
                nc.vector.tensor_tensor_scan(
                    out=pflat(g_t), data0=invpseg_t[:], data1=pflat(fs_t),
                    initial=0.0, op0=Alu.mult, op1=Alu.max)

                nc.gpsimd.tensor_single_scalar(
                    out=sgrp[pp][:, :, :, go], in_=g_t[:],
                    scalar=1.0, op=Alu.is_equal)
                memp[pp] = mem_t
                gp[pp] = g_t

                if n == 0 and pp == 0:
                    # phase-offset the two pipelines: Act executes strictly
                    # in-order, so this op (gated on pipeline 0's first g)
                    # delays pipeline 1's first x-copy by ~half a period
                    ph_t = small.tile([128, 1], f32, tag="ph")
                    nc.scalar.mul(ph_t[:], g_t[:, 0, 0, 0:1], 1.0)

                if not last:
                    glast = g_t[:, :, :, T - 1:T]
                    memlast = mem_t[:, :, :, T - 1:T]
                    ns_t = small.tile(pcast, f32, tag=f"ns{pp}",
                                      name=f"ns{pp}")
                    nc.vector.tensor_scalar(out=ns_t[:], in0=glast,
                                            scalar1=1.0, scalar2=None,
                                            op0=Alu.is_lt)
                    vin_t = small.tile(pcast, f32, tag=f"vin{pp}",
                                       name=f"vin{pp}")
                    nc.vector.tensor_tensor(out=vin_t[:], in0=memlast,
                                            in1=ns_t[:], op=Alu.mult)
                    vb_new = small.tile(pcast, f32, tag=f"vb{pp}",
                                        name=f"vb{pp}")
                    nc.vector.tensor_tensor(
                        out=vb_new[:], in0=vin_t[:],
                        in1=betat_t.broadcast_to(pcast), op=Alu.mult)
                    q_new = work.tile(pfull, f32, tag=f"q{pp}", name=f"q{pp}")
                    nc.gpsimd.tensor_tensor(
                        out=q_new[:], in0=g_t[:],
                        in1=ns_t.broadcast_to(pfull), op=Alu.add)

                    # adaptation chain (tiny [128, CH*BP] ops): mask the
                    # no-spike lanes by swamping g_last before the reciprocal
                    gs_t = small.tile(pcast, f32, tag=f"gs{pp}",
                                      name=f"gs{pp}")
                    nc.vector.scalar_tensor_tensor(
                        out=gs_t[:], in0=ns_t[:], scalar=1e30, in1=glast,
                        op0=Alu.mult, op1=Alu.add)
                    gr_t = small.tile(pcast, f32, tag=f"gr{pp}",
                                      name=f"gr{pp}")
                    nc.vector.reciprocal(out=gr_t[:], in_=gs_t[:])
                    ua_t = small.tile(pcast, f32, tag=f"ua{pp}",
                                      name=f"ua{pp}")
                    nc.gpsimd.tensor_tensor(
                        out=ua_t[:], in0=gr_t[:],
                        in1=invp_t.broadcast_to(pcast), op=Alu.mult)
                    av_t = small.tile(pcast, f32, tag=f"av{pp}",
                                      name=f"av{pp}")
                    nc.gpsimd.tensor_tensor(
                        out=av_t[:], in0=(a_prev[pp] if n > 0 else a0_t)[:],
                        in1=p32_t.broadcast_to(pcast), op=Alu.mult)
                    an_t = small.tile(pcast, f32, tag=f"an{pp}",
                                      name=f"an{pp}")
                    nc.gpsimd.tensor_tensor(out=an_t[:], in0=av_t[:],
                                            in1=ua_t[:], op=Alu.add)

                    q_prev[pp] = q_new
                    a_prev[pp] = an_t
                    vb_prev[pp] = vb_new

            if go == GRP - 1:
                HS = CH * BP * GRP * T
                for pp in range(NP):
                    dma.dma_start(
                        out=bass.AP(tensor=out_d,
                                    offset=(gi * NP + pp) * 128 * HS,
                                    ap=[[HS, 128], [1, HS]]),
                        in_=sgrp[pp][:].rearrange("p c b g t -> p (c b g t)"))
            go_prev = go

    with tile.TileContext(nc) as tc:
        kern(tc)
    nc.compile()
    return nc


def _host_tables(beta_raw, rec_weight, p_raw, b_raw):
    f = np.float32
    W = rec_weight.astype(f)
    beta = np.clip(beta_raw.astype(f), f(0.001), f(0.999))
    p = np.clip(np.abs(p_raw.astype(f)), f(0.0), f(0.999))
    bb = np.clip(np.abs(b_raw.astype(f)), f(0.001), f(1.0))
    p_pow = (p[:, None] ** np.arange(1, T + 1, dtype=f)).astype(f)   # (C,T)
    BP1 = (bb[:, None] * p_pow).astype(f)
    p32 = np.ascontiguousarray(p_pow[:, -1])
    invp = (f(1.0) / p).astype(f)

    def per_ct(vals_ct, nb):  # (C,T) -> (128, CH*nb*T), replicated over b
        v = vals_ct.reshape(CH, 128, T)
        out = np.zeros((128, CH, nb, T), f)
        out[:] = v.transpose(1, 0, 2)[:, :, None, :]
        return np.ascontiguousarray(out.reshape(128, CH * nb * T))

    t0mask = np.ones((1, T), f)
    t0mask[0, 0] = 0.0
    betaseg = per_ct((beta[:, None] * t0mask).astype(f), BP)
    invpseg = per_ct((invp[:, None] * t0mask).astype(f), BP)
    bp1 = per_ct(BP1, BP)

    def per_c(vals_c):  # (C,) -> (128, CH)
        return np.ascontiguousarray(vals_c.reshape(CH, 128).T)

    # wt[v*16 + cj_hi*CH + ci_hi][cj_lo, ci_lo] = split_v[ci_hi*128+ci_lo, cj_hi*128+cj_lo]
    import ml_dtypes
    W4 = W.reshape(CH, 128, CH, 128)
    wt16 = np.ascontiguousarray(
        W4.transpose(2, 0, 3, 1).reshape(16, 128, 128))
    # exact 3-way bf16 decomposition: w1+w2+w3 == W to ~2^-27 relative
    w1 = wt16.astype(ml_dtypes.bfloat16)
    r1 = wt16 - w1.astype(f)
    w2 = r1.astype(ml_dtypes.bfloat16)
    r2 = r1 - w2.astype(f)
    w3 = r2.astype(ml_dtypes.bfloat16)
    wt = np.ascontiguousarray(np.concatenate([w1, w2, w3], axis=0))
    ident = np.eye(128, dtype=f)
    return dict(wt=wt, ident=ident, betaseg=betaseg, invpseg=invpseg, bp1=bp1,
                betat=per_c(beta), p32t=per_c(p32), invpt=per_c(invp))


def kernel(x, beta_raw, rec_weight, p_raw, b_raw):
    global _compiled
    from concourse.bass_utils import run_bass_kernel_spmd

    if _compiled is None:
        _compiled = _build_program()
    nc = _compiled

    tables = _host_tables(np.asarray(beta_raw), np.asarray(rec_weight),
                          np.asarray(p_raw), np.asarray(b_raw))
    x = np.asarray(x).astype(np.float32)
    in_maps = []
    for k in range(NCORES):
        xc = x[k * BPC:(k + 1) * BPC]                       # (BPC, C, T_LEN)
        xt = xc.reshape(BPC, CH, 128, NB, T).transpose(3, 2, 1, 0, 4)
        m = {"xt": np.ascontiguousarray(xt)}                # (NB,128,CH,BPC,T)
        m.update(tables)
        in_maps.append(m)
    res = run_bass_kernel_spmd(nc, in_maps, list(range(NCORES)))
    out = np.empty((B, C, T_LEN), np.float32)
    for k in range(NCORES):
        og = np.asarray(res.results[k]["out"]).astype(np.float32)
        og = og.reshape(NG, NP, 128, CH, BP, GRP, T).transpose(1, 4, 3, 2, 0, 5, 6)
        out[k * BPC:(k + 1) * BPC] = og.reshape(BPC, C, T_LEN)
    return out
